# revision 13
# baseline (speedup 1.0000x reference)
"""Single-head attention (B=4, S=2048, E=1024) on 8 TRN2 NeuronCores.

Sharding: core c -> (batch b = c//2, query-half h = c%2). Each core
computes K/V projections for its whole batch (duplicated across the
core pair -> no collectives), Q for its 1024-row query chunk, then
scores^T, softmax (no max-subtraction; logits are in [-6, 6], a fixed
-4 shift cancels exactly in the normalization), and P@V.

All matmul operands keep the contraction dim on partitions, so no
on-device transposes are needed anywhere:
  Q^T[f,q]  = sum_e WqT[e,f]^T xqT[e,q]      (stationary WqT tile)
  K^T[f,k]  = sum_e WkT[e,f]^T xT[e,k]
  V[k,f]    = sum_e xT[e,k]^T  WvT[e,f]      (stationary x tile)
  S^T[k,q]  = sum_f KT[f,k]^T  QT[f,q]
  P         = exp(S^T/32 - 4)                 (ScalarE, PSUM->SBUF bf16)
  den[1,q]  = sum_k ones[k,1]^T P[k,q]        (PE cross-partition sum)
  O^T[e,q]  = sum_k V[k,e]^T   P[k,q]
  out       = O^T * (1/den) broadcast         (VectorE, fp32)

Compute dtype bf16 with fp32 PSUM accumulation (numpy model of this
exact pipeline: absmax err / out scale ~= 6e-3).
"""

import numpy as np
import ml_dtypes

import concourse.bass as bass
import concourse.tile as tile
from concourse import bacc, mybir
from concourse.bass_utils import run_bass_kernel_spmd

B, S, E = 4, 2048, 1024
N_CORES = 8
SQ = S // 2        # query rows per core
P = 128            # SBUF partitions
NT = 512           # matmul moving-operand slice (one PSUM bank fp32)
ET = E // P        # 8 feature/contraction tiles
KT = S // P        # 16 key tiles
FP32 = mybir.dt.float32
BF16 = mybir.dt.bfloat16
SCALE = 1.0 / np.sqrt(E).astype(np.float32)
SHIFT = -4.0       # constant logit shift; cancels in softmax


STAGES = 5  # dev bisect knob: 1=projections 2=+scores/exp 3=+sums 4=PV-unnormalized 5=full


def build_kernel(ctx, tc, io):
    nc = tc.nc
    xT, xqT, wqT, wkT, wvT, bq, bk, bv, outT = (
        io["xT"], io["xqT"], io["wqT"], io["wkT"], io["wvT"],
        io["bq"], io["bk"], io["bv"], io["outT"],
    )

    singles = ctx.enter_context(tc.tile_pool(name="singles", bufs=1))
    results = ctx.enter_context(tc.tile_pool(name="results", bufs=1))
    # x_sb is dead once the V projection has consumed it; p_sb is written
    # strictly after that (PE program order), so they share one 32KB slot.
    xp_pool = ctx.enter_context(tc.tile_pool(name="xp", bufs=1))
    outp = ctx.enter_context(tc.tile_pool(name="outp", bufs=3))
    ps_main = ctx.enter_context(tc.tile_pool(name="ps_main", bufs=3, space="PSUM"))
    ps_sums = ctx.enter_context(tc.tile_pool(name="ps_sums", bufs=2, space="PSUM"))
    ps_out = ctx.enter_context(tc.tile_pool(name="ps_out", bufs=2, space="PSUM"))

    # ---- input staging (one DMA per 128-partition slab -> parallel queues)
    wq_sb = singles.tile([P, ET, E], BF16)
    wk_sb = singles.tile([P, ET, E], BF16)
    wv_sb = singles.tile([P, ET, E], BF16)
    xq_sb = singles.tile([P, ET, SQ], BF16)
    x_sb = xp_pool.tile([P, ET, S], BF16, tag="xp")
    for t in range(ET):
        r = slice(t * P, (t + 1) * P)
        nc.sync.dma_start(out=wq_sb[:, t, :], in_=wqT[r, :])
        nc.sync.dma_start(out=xq_sb[:, t, :], in_=xqT[r, :])
        nc.sync.dma_start(out=wk_sb[:, t, :], in_=wkT[r, :])
        nc.sync.dma_start(out=x_sb[:, t, :], in_=xT[r, :])
        nc.sync.dma_start(out=wv_sb[:, t, :], in_=wvT[r, :])

    bq_sb = singles.tile([P, ET], FP32)
    bk_sb = singles.tile([P, ET], FP32)
    nc.sync.dma_start(out=bq_sb, in_=bq.rearrange("(t p) -> p t", p=P))
    nc.sync.dma_start(out=bk_sb, in_=bk.rearrange("(t p) -> p t", p=P))
    bv_bc = singles.tile([P, E], FP32)
    nc.sync.dma_start(out=bv_bc, in_=bv.partition_broadcast(P))

    ones_sb = singles.tile([P, 1], BF16)
    nc.vector.memset(ones_sb, 1.0)
    shift_sb = singles.tile([P, 1], FP32)
    nc.vector.memset(shift_sb, SHIFT)

    qT_sb = results.tile([P, ET, SQ], BF16)
    kT_sb = results.tile([P, ET, S], BF16)
    v_sb = results.tile([P, KT, E], BF16)
    p_sb = xp_pool.tile([P, KT, SQ], BF16, tag="xp")

    ident = mybir.ActivationFunctionType.Identity

    # ---- Q^T = Wq x_q + bq   [f, q]
    for ft in range(ET):
        fr = slice(ft * P, (ft + 1) * P)
        for qs in range(SQ // NT):
            qr = slice(qs * NT, (qs + 1) * NT)
            ps = ps_main.tile([P, NT], FP32)
            for et in range(ET):
                nc.tensor.matmul(ps, lhsT=wq_sb[:, et, fr], rhs=xq_sb[:, et, qr],
                                 start=(et == 0), stop=(et == ET - 1))
            nc.scalar.activation(out=qT_sb[:, ft, qr], in_=ps, func=ident,
                                 bias=bq_sb[:, ft:ft + 1], scale=1.0)

    # ---- K^T = Wk x + bk   [f, k]
    for ft in range(ET):
        fr = slice(ft * P, (ft + 1) * P)
        for ks in range(S // NT):
            kr = slice(ks * NT, (ks + 1) * NT)
            ps = ps_main.tile([P, NT], FP32)
            for et in range(ET):
                nc.tensor.matmul(ps, lhsT=wk_sb[:, et, fr], rhs=x_sb[:, et, kr],
                                 start=(et == 0), stop=(et == ET - 1))
            nc.scalar.activation(out=kT_sb[:, ft, kr], in_=ps, func=ident,
                                 bias=bk_sb[:, ft:ft + 1], scale=1.0)

    # ---- V = x Wv^T + bv   [k, f]  (bias along free dim -> broadcast add)
    for kt in range(KT):
        kr = slice(kt * P, (kt + 1) * P)
        for fs in range(E // NT):
            fr = slice(fs * NT, (fs + 1) * NT)
            ps = ps_main.tile([P, NT], FP32)
            for et in range(ET):
                nc.tensor.matmul(ps, lhsT=x_sb[:, et, kr], rhs=wv_sb[:, et, fr],
                                 start=(et == 0), stop=(et == ET - 1))
            nc.vector.tensor_add(v_sb[:, kt, fr], ps, bv_bc[:, fr])

    if STAGES < 2:
        return
    # ---- scores^T and P = exp(S^T * scale + shift)   [k, q]
    for kt in range(KT):
        kr = slice(kt * P, (kt + 1) * P)
        for qs in range(SQ // NT):
            qr = slice(qs * NT, (qs + 1) * NT)
            ps = ps_main.tile([P, NT], FP32)
            for et in range(ET):
                nc.tensor.matmul(ps, lhsT=kT_sb[:, et, kr], rhs=qT_sb[:, et, qr],
                                 start=(et == 0), stop=(et == ET - 1))
            nc.scalar.activation(out=p_sb[:, kt, qr], in_=ps,
                                 func=mybir.ActivationFunctionType.Exp,
                                 bias=shift_sb[:, 0:1], scale=float(SCALE))

    if STAGES < 3:
        return
    # ---- denominators: den[1, q] = sum_k P[k, q]  (PE ones-matmul)
    recip_sb = singles.tile([1, SQ], FP32)
    for qs in range(SQ // NT):
        qr = slice(qs * NT, (qs + 1) * NT)
        ps = ps_sums.tile([1, NT], FP32)
        for kt in range(KT):
            nc.tensor.matmul(ps, lhsT=ones_sb, rhs=p_sb[:, kt, qr],
                             start=(kt == 0), stop=(kt == KT - 1))
        nc.vector.reciprocal(out=recip_sb[:, qr], in_=ps)

    # broadcast 1/den across partitions (DRAM bounce; stride-0 partition
    # reads are only legal from DRAM)
    dram = ctx.enter_context(tc.tile_pool(name="dram", bufs=1, space="DRAM"))
    recip_dram = dram.tile([1, SQ], FP32)
    nc.sync.dma_start(out=recip_dram, in_=recip_sb)
    recip_bc = singles.tile([P, SQ], FP32)
    nc.sync.dma_start(out=recip_bc, in_=recip_dram[0, :].partition_broadcast(P))

    if STAGES < 4:
        return
    # ---- O^T = V^T P, then normalize rows  [e, q]
    for ft in range(ET):
        fr = slice(ft * P, (ft + 1) * P)
        for qs in range(SQ // NT):
            qr = slice(qs * NT, (qs + 1) * NT)
            ps = ps_out.tile([P, NT], FP32)
            for kt in range(KT):
                nc.tensor.matmul(ps, lhsT=v_sb[:, kt, fr], rhs=p_sb[:, kt, qr],
                                 start=(kt == 0), stop=(kt == KT - 1))
            ot = outp.tile([P, NT], FP32)
            if STAGES < 5:
                nc.vector.tensor_copy(out=ot, in_=ps)
            else:
                nc.vector.tensor_mul(ot, ps, recip_bc[:, qr])
            nc.sync.dma_start(out=outT[fr, qr], in_=ot)


def build_program():
    nc = bacc.Bacc("TRN2", target_bir_lowering=False, debug=False,
                   num_devices=N_CORES)
    io = {
        "xT": nc.dram_tensor("xT", [E, S], BF16, kind="ExternalInput").ap(),
        "xqT": nc.dram_tensor("xqT", [E, SQ], BF16, kind="ExternalInput").ap(),
        "wqT": nc.dram_tensor("wqT", [E, E], BF16, kind="ExternalInput").ap(),
        "wkT": nc.dram_tensor("wkT", [E, E], BF16, kind="ExternalInput").ap(),
        "wvT": nc.dram_tensor("wvT", [E, E], BF16, kind="ExternalInput").ap(),
        "bq": nc.dram_tensor("bq", [E], FP32, kind="ExternalInput").ap(),
        "bk": nc.dram_tensor("bk", [E], FP32, kind="ExternalInput").ap(),
        "bv": nc.dram_tensor("bv", [E], FP32, kind="ExternalInput").ap(),
        "outT": nc.dram_tensor("outT", [E, SQ], FP32, kind="ExternalOutput").ap(),
    }
    from contextlib import ExitStack
    with tile.TileContext(nc) as tc:
        with ExitStack() as ctx:
            build_kernel(ctx, tc, io)
    nc.compile()
    return nc


def make_in_maps(x, wq_w, wq_b, wk_w, wk_b, wv_w, wv_b):
    bf = ml_dtypes.bfloat16
    xT_all = np.ascontiguousarray(np.transpose(np.asarray(x, np.float32),
                                               (0, 2, 1))).astype(bf)
    wqT = np.ascontiguousarray(np.asarray(wq_w, np.float32).T).astype(bf)
    wkT = np.ascontiguousarray(np.asarray(wk_w, np.float32).T).astype(bf)
    wvT = np.ascontiguousarray(np.asarray(wv_w, np.float32).T).astype(bf)
    bq = np.asarray(wq_b, np.float32)
    bk = np.asarray(wk_b, np.float32)
    bv = np.asarray(wv_b, np.float32)
    in_maps = []
    for c in range(N_CORES):
        b, h = divmod(c, 2)
        in_maps.append({
            "xT": xT_all[b],
            "xqT": np.ascontiguousarray(xT_all[b][:, h * SQ:(h + 1) * SQ]),
            "wqT": wqT, "wkT": wkT, "wvT": wvT,
            "bq": bq, "bk": bk, "bv": bv,
        })
    return in_maps


def assemble_out(results):
    out = np.empty((B, S, E), np.float32)
    for c in range(N_CORES):
        b, h = divmod(c, 2)
        out[b, h * SQ:(h + 1) * SQ, :] = results[c]["outT"].T
    return out


_NC_CACHE = None


def kernel(x, wq_w, wq_b, wk_w, wk_b, wv_w, wv_b):
    global _NC_CACHE
    if _NC_CACHE is None:
        _NC_CACHE = build_program()
    in_maps = make_in_maps(x, wq_w, wq_b, wk_w, wk_b, wv_w, wv_b)
    res = run_bass_kernel_spmd(_NC_CACHE, in_maps, list(range(N_CORES)))
    return assemble_out(res.results)


# revision 14
# speedup vs baseline: 1.3359x; 1.3359x over previous
"""Single-head attention (B=4, S=2048, E=1024) on 8 TRN2 NeuronCores.

Sharding: core c -> (batch b = c//2, query-half h = c%2). Each core
computes K/V projections for its whole batch (duplicated across the
core pair -> no collectives), Q for its 1024-row query chunk, then
scores^T, softmax (no max-subtraction; logits are in [-6, 6], a fixed
-4 shift cancels exactly in the normalization), and P@V.

All matmul operands keep the contraction dim on partitions, so no
on-device transposes are needed anywhere:
  Q^T[f,q]  = sum_e WqT[e,f]^T xqT[e,q]      (stationary WqT tile)
  K^T[f,k]  = sum_e WkT[e,f]^T xT[e,k]
  V[k,f]    = sum_e xT[e,k]^T  WvT[e,f]      (stationary x tile)
  S^T[k,q]  = sum_f KT[f,k]^T  QT[f,q]
  P         = exp(S^T/32 - 4)                 (ScalarE, PSUM->SBUF bf16)
  den[1,q]  = sum_k ones[k,1]^T P[k,q]        (PE cross-partition sum)
  O^T[e,q]  = sum_k V[k,e]^T   P[k,q]
  out       = O^T * (1/den) broadcast         (VectorE, fp32)

Compute dtype bf16 with fp32 PSUM accumulation (numpy model of this
exact pipeline: absmax err / out scale ~= 6e-3).
"""

import numpy as np
import ml_dtypes

import concourse.bass as bass
import concourse.tile as tile
from concourse import bacc, mybir
from concourse.bass_utils import run_bass_kernel_spmd

B, S, E = 4, 2048, 1024
N_CORES = 8
SQ = S // 2        # query rows per core
P = 128            # SBUF partitions
NT = 512           # matmul moving-operand slice (one PSUM bank fp32)
ET = E // P        # 8 feature/contraction tiles
KT = S // P        # 16 key tiles
FP32 = mybir.dt.float32
BF16 = mybir.dt.bfloat16
SCALE = 1.0 / np.sqrt(E).astype(np.float32)
SHIFT = -4.0       # constant logit shift; cancels in softmax


STAGES = 5  # dev bisect knob: 1=projections 2=+scores/exp 3=+sums 4=PV-unnormalized 5=full


def build_kernel(ctx, tc, io):
    nc = tc.nc
    xT, xqT, wqT, wkT, wvT, bq, bk, bv, outT = (
        io["xT"], io["xqT"], io["wqT"], io["wkT"], io["wvT"],
        io["bq"], io["bk"], io["bv"], io["outT"],
    )

    singles = ctx.enter_context(tc.tile_pool(name="singles", bufs=1))
    results = ctx.enter_context(tc.tile_pool(name="results", bufs=1))
    # x_sb is dead once the V projection has consumed it; p_sb is written
    # strictly after that (PE program order), so they share one 32KB slot.
    xp_pool = ctx.enter_context(tc.tile_pool(name="xp", bufs=1))
    outp = ctx.enter_context(tc.tile_pool(name="outp", bufs=3))
    ps_main = ctx.enter_context(tc.tile_pool(name="ps_main", bufs=3, space="PSUM"))
    ps_sums = ctx.enter_context(tc.tile_pool(name="ps_sums", bufs=2, space="PSUM"))
    ps_out = ctx.enter_context(tc.tile_pool(name="ps_out", bufs=2, space="PSUM"))

    # ---- input staging (one DMA per 128-partition slab -> parallel queues)
    wq_sb = singles.tile([P, ET, E], BF16)
    wk_sb = singles.tile([P, ET, E], BF16)
    wv_sb = singles.tile([P, ET, E], BF16)
    xq_sb = singles.tile([P, ET, SQ], BF16)
    x_sb = xp_pool.tile([P, ET, S], BF16, tag="xp")
    # Stream inputs in first-use order: Q-projection operands first so the
    # PE can start ~11us in; K/V operands arrive while Q-proj computes.
    for t in range(ET):
        r = slice(t * P, (t + 1) * P)
        nc.sync.dma_start(out=wq_sb[:, t, :], in_=wqT[r, :])
        nc.sync.dma_start(out=xq_sb[:, t, :], in_=xqT[r, :])
    for t in range(ET):
        r = slice(t * P, (t + 1) * P)
        nc.sync.dma_start(out=wk_sb[:, t, :], in_=wkT[r, :])
        nc.sync.dma_start(out=x_sb[:, t, :], in_=xT[r, :])
    for t in range(ET):
        r = slice(t * P, (t + 1) * P)
        nc.sync.dma_start(out=wv_sb[:, t, :], in_=wvT[r, :])

    bq_sb = singles.tile([P, ET], FP32)
    bk_sb = singles.tile([P, ET], FP32)
    nc.sync.dma_start(out=bq_sb, in_=bq.rearrange("(t p) -> p t", p=P))
    nc.sync.dma_start(out=bk_sb, in_=bk.rearrange("(t p) -> p t", p=P))
    bv_bc = singles.tile([P, E], FP32)
    nc.sync.dma_start(out=bv_bc, in_=bv.partition_broadcast(P))

    ones_sb = singles.tile([P, 1], BF16)
    nc.vector.memset(ones_sb, 1.0)
    shift_sb = singles.tile([P, 1], FP32)
    nc.vector.memset(shift_sb, SHIFT)

    qT_sb = results.tile([P, ET, SQ], BF16)
    kT_sb = results.tile([P, ET, S], BF16)
    v_sb = results.tile([P, KT, E], BF16)
    p_sb = xp_pool.tile([P, KT, SQ], BF16, tag="xp")

    ident = mybir.ActivationFunctionType.Identity

    # ---- Q^T = Wq x_q + bq   [f, q]
    for ft in range(ET):
        fr = slice(ft * P, (ft + 1) * P)
        for qs in range(SQ // NT):
            qr = slice(qs * NT, (qs + 1) * NT)
            ps = ps_main.tile([P, NT], FP32)
            for et in range(ET):
                nc.tensor.matmul(ps, lhsT=wq_sb[:, et, fr], rhs=xq_sb[:, et, qr],
                                 start=(et == 0), stop=(et == ET - 1))
            nc.scalar.activation(out=qT_sb[:, ft, qr], in_=ps, func=ident,
                                 bias=bq_sb[:, ft:ft + 1], scale=1.0)

    # ---- K^T = Wk x + bk   [f, k]
    for ft in range(ET):
        fr = slice(ft * P, (ft + 1) * P)
        for ks in range(S // NT):
            kr = slice(ks * NT, (ks + 1) * NT)
            ps = ps_main.tile([P, NT], FP32)
            for et in range(ET):
                nc.tensor.matmul(ps, lhsT=wk_sb[:, et, fr], rhs=x_sb[:, et, kr],
                                 start=(et == 0), stop=(et == ET - 1))
            nc.scalar.activation(out=kT_sb[:, ft, kr], in_=ps, func=ident,
                                 bias=bk_sb[:, ft:ft + 1], scale=1.0)

    # ---- V = x Wv^T + bv   [k, f]  (bias along free dim -> broadcast add)
    for kt in range(KT):
        kr = slice(kt * P, (kt + 1) * P)
        for fs in range(E // NT):
            fr = slice(fs * NT, (fs + 1) * NT)
            ps = ps_main.tile([P, NT], FP32)
            for et in range(ET):
                nc.tensor.matmul(ps, lhsT=x_sb[:, et, kr], rhs=wv_sb[:, et, fr],
                                 start=(et == 0), stop=(et == ET - 1))
            nc.vector.tensor_add(v_sb[:, kt, fr], ps, bv_bc[:, fr])

    if STAGES < 2:
        return
    # ---- scores^T and P = exp(S^T * scale + shift)   [k, q]
    for kt in range(KT):
        kr = slice(kt * P, (kt + 1) * P)
        for qs in range(SQ // NT):
            qr = slice(qs * NT, (qs + 1) * NT)
            ps = ps_main.tile([P, NT], FP32)
            for et in range(ET):
                nc.tensor.matmul(ps, lhsT=kT_sb[:, et, kr], rhs=qT_sb[:, et, qr],
                                 start=(et == 0), stop=(et == ET - 1))
            nc.scalar.activation(out=p_sb[:, kt, qr], in_=ps,
                                 func=mybir.ActivationFunctionType.Exp,
                                 bias=shift_sb[:, 0:1], scale=float(SCALE))

    if STAGES < 3:
        return
    # ---- denominators: den[1, q] = sum_k P[k, q]  (PE ones-matmul)
    recip_sb = singles.tile([1, SQ], FP32)
    for qs in range(SQ // NT):
        qr = slice(qs * NT, (qs + 1) * NT)
        ps = ps_sums.tile([1, NT], FP32)
        for kt in range(KT):
            nc.tensor.matmul(ps, lhsT=ones_sb, rhs=p_sb[:, kt, qr],
                             start=(kt == 0), stop=(kt == KT - 1))
        nc.vector.reciprocal(out=recip_sb[:, qr], in_=ps)

    # broadcast 1/den across partitions (DRAM bounce; stride-0 partition
    # reads are only legal from DRAM)
    dram = ctx.enter_context(tc.tile_pool(name="dram", bufs=1, space="DRAM"))
    recip_dram = dram.tile([1, SQ], FP32)
    nc.sync.dma_start(out=recip_dram, in_=recip_sb)
    recip_bc = singles.tile([P, SQ], FP32)
    nc.sync.dma_start(out=recip_bc, in_=recip_dram[0, :].partition_broadcast(P))

    if STAGES < 4:
        return
    # ---- O^T = V^T P, then normalize rows  [e, q]
    for ft in range(ET):
        fr = slice(ft * P, (ft + 1) * P)
        for qs in range(SQ // NT):
            qr = slice(qs * NT, (qs + 1) * NT)
            ps = ps_out.tile([P, NT], FP32)
            for kt in range(KT):
                nc.tensor.matmul(ps, lhsT=v_sb[:, kt, fr], rhs=p_sb[:, kt, qr],
                                 start=(kt == 0), stop=(kt == KT - 1))
            ot = outp.tile([P, NT], FP32)
            if STAGES < 5:
                nc.vector.tensor_copy(out=ot, in_=ps)
            else:
                nc.vector.tensor_mul(ot, ps, recip_bc[:, qr])
            nc.sync.dma_start(out=outT[fr, qr], in_=ot)


def build_program():
    nc = bacc.Bacc("TRN2", target_bir_lowering=False, debug=False,
                   num_devices=N_CORES)
    io = {
        "xT": nc.dram_tensor("xT", [E, S], BF16, kind="ExternalInput").ap(),
        "xqT": nc.dram_tensor("xqT", [E, SQ], BF16, kind="ExternalInput").ap(),
        "wqT": nc.dram_tensor("wqT", [E, E], BF16, kind="ExternalInput").ap(),
        "wkT": nc.dram_tensor("wkT", [E, E], BF16, kind="ExternalInput").ap(),
        "wvT": nc.dram_tensor("wvT", [E, E], BF16, kind="ExternalInput").ap(),
        "bq": nc.dram_tensor("bq", [E], FP32, kind="ExternalInput").ap(),
        "bk": nc.dram_tensor("bk", [E], FP32, kind="ExternalInput").ap(),
        "bv": nc.dram_tensor("bv", [E], FP32, kind="ExternalInput").ap(),
        "outT": nc.dram_tensor("outT", [E, SQ], FP32, kind="ExternalOutput").ap(),
    }
    from contextlib import ExitStack
    with tile.TileContext(nc) as tc:
        with ExitStack() as ctx:
            build_kernel(ctx, tc, io)
    nc.compile()
    return nc


def make_in_maps(x, wq_w, wq_b, wk_w, wk_b, wv_w, wv_b):
    bf = ml_dtypes.bfloat16
    xT_all = np.ascontiguousarray(np.transpose(np.asarray(x, np.float32),
                                               (0, 2, 1))).astype(bf)
    wqT = np.ascontiguousarray(np.asarray(wq_w, np.float32).T).astype(bf)
    wkT = np.ascontiguousarray(np.asarray(wk_w, np.float32).T).astype(bf)
    wvT = np.ascontiguousarray(np.asarray(wv_w, np.float32).T).astype(bf)
    bq = np.asarray(wq_b, np.float32)
    bk = np.asarray(wk_b, np.float32)
    bv = np.asarray(wv_b, np.float32)
    in_maps = []
    for c in range(N_CORES):
        b, h = divmod(c, 2)
        in_maps.append({
            "xT": xT_all[b],
            "xqT": np.ascontiguousarray(xT_all[b][:, h * SQ:(h + 1) * SQ]),
            "wqT": wqT, "wkT": wkT, "wvT": wvT,
            "bq": bq, "bk": bk, "bv": bv,
        })
    return in_maps


def assemble_out(results):
    out = np.empty((B, S, E), np.float32)
    for c in range(N_CORES):
        b, h = divmod(c, 2)
        out[b, h * SQ:(h + 1) * SQ, :] = results[c]["outT"].T
    return out


_NC_CACHE = None


def kernel(x, wq_w, wq_b, wk_w, wk_b, wv_w, wv_b):
    global _NC_CACHE
    if _NC_CACHE is None:
        _NC_CACHE = build_program()
    in_maps = make_in_maps(x, wq_w, wq_b, wk_w, wk_b, wv_w, wv_b)
    res = run_bass_kernel_spmd(_NC_CACHE, in_maps, list(range(N_CORES)))
    return assemble_out(res.results)
